# revision 1
# baseline (speedup 1.0000x reference)
"""Data-parallel Trainium2 kernel for nn_EnrichedNodeHead.

Shards the node dimension N=131072 across 8 NeuronCores (pure data
parallel, weights replicated), computes the per-node head on each core,
and gathers the full (N, 8) output.
"""

import numpy as np
import jax
import jax.numpy as jnp

N = 131072
D = 64
H = 4
NCI = 10
NCLS = 8
NDEV = 8

_WNAMES = [
    "W_in", "b_in", "W_out", "b_out", "g_attn", "b_attn",
    "Wi1", "bi1", "Wi2", "bi2", "gi", "bni",
    "Wc1", "bc1", "Wc2", "bc2", "gc", "bnc",
    "Wm", "bm", "gm", "bnm",
    "Wk1", "bk1", "Wk2", "bk2",
]


def _ln(x, g, b, eps=1e-5):
    mu = x.mean(-1, keepdims=True)
    var = ((x - mu) ** 2).mean(-1, keepdims=True)
    return (x - mu) / jnp.sqrt(var + eps) * g + b


def _gelu(x):
    return jax.nn.gelu(x, approximate=False)


def _shard_fn(e_vx, e_vy, e_xv, e_yv, ci_features,
              W_in, b_in, W_out, b_out, g_attn, b_attn,
              Wi1, bi1, Wi2, bi2, gi, bni,
              Wc1, bc1, Wc2, bc2, gc, bnc,
              Wm, bm, gm, bnm,
              Wk1, bk1, Wk2, bk2):
    n = e_vx.shape[0]
    hd = D // H
    edges = jnp.stack([e_vx, e_vy, e_xv, e_yv], axis=1)        # (n,4,D)
    qkv = edges @ W_in.T + b_in                                # (n,4,3D)
    q, k, v = jnp.split(qkv, 3, axis=-1)
    sh = lambda t: t.reshape(n, 4, H, hd).transpose(0, 2, 1, 3)
    q, k, v = sh(q), sh(k), sh(v)
    scores = jnp.einsum("nhqe,nhke->nhqk", q, k) * (1.0 / hd ** 0.5)
    att = jax.nn.softmax(scores, axis=-1)
    ao = jnp.einsum("nhqk,nhke->nhqe", att, v).transpose(0, 2, 1, 3).reshape(n, 4, D)
    attended = ao @ W_out.T + b_out
    attended = _ln(edges + attended, g_attn, b_attn)
    pooled = attended.mean(axis=1)
    inter = jnp.concatenate(
        [e_vx * e_vy, e_vx * e_xv, e_vx * e_yv,
         e_vy * e_xv, e_vy * e_yv, e_xv * e_yv], axis=-1)
    interaction_emb = _ln(_gelu(inter @ Wi1.T + bi1) @ Wi2.T + bi2, gi, bni)
    ci_emb = _ln(_gelu(ci_features @ Wc1.T + bc1) @ Wc2.T + bc2, gc, bnc)
    merged = _gelu(_ln(
        jnp.concatenate([pooled, interaction_emb, ci_emb], axis=-1) @ Wm.T + bm,
        gm, bnm))
    return _gelu(merged @ Wk1.T + bk1) @ Wk2.T + bk2


_pmapped = None


def _get_pmapped():
    global _pmapped
    if _pmapped is None:
        _pmapped = jax.pmap(_shard_fn, devices=jax.devices()[:NDEV])
    return _pmapped


def kernel(**inputs):
    fn = _get_pmapped()
    shard = lambda a: np.asarray(a, dtype=np.float32).reshape(
        NDEV, -1, *a.shape[1:])
    rep = lambda a: np.broadcast_to(
        np.asarray(a, dtype=np.float32), (NDEV,) + np.asarray(a).shape)
    args = [shard(inputs[k]) for k in
            ("e_vx", "e_vy", "e_xv", "e_yv", "ci_features")]
    args += [rep(inputs[k]) for k in _WNAMES]
    out = fn(*args)
    return np.asarray(out).reshape(N, NCLS)


# revision 2
# speedup vs baseline: 1.0051x; 1.0051x over previous
"""Data-parallel Trainium2 kernel for nn_EnrichedNodeHead.

Shards the node dimension N=131072 across 8 NeuronCores (pure data
parallel, weights replicated), computes the per-node head on each core,
and gathers the full (N, 8) output.

Transfer strategy: the five per-node tensors are packed host-side into a
single (8, N/8, 266) array and the 26 weight/bias tensors into a single
flat vector, so each call ships exactly two host->device arrays instead
of 31x8. Weight uploads are cached across calls keyed on array identity.
"""

import numpy as np
import jax
import jax.numpy as jnp

N = 131072
D = 64
H = 4
NCI = 10
NCLS = 8
NDEV = 8
PACKC = 4 * D + NCI  # 266

_WNAMES = [
    "W_in", "b_in", "W_out", "b_out", "g_attn", "b_attn",
    "Wi1", "bi1", "Wi2", "bi2", "gi", "bni",
    "Wc1", "bc1", "Wc2", "bc2", "gc", "bnc",
    "Wm", "bm", "gm", "bnm",
    "Wk1", "bk1", "Wk2", "bk2",
]
_WSHAPES = {
    "W_in": (3 * D, D), "b_in": (3 * D,), "W_out": (D, D), "b_out": (D,),
    "g_attn": (D,), "b_attn": (D,),
    "Wi1": (2 * D, 6 * D), "bi1": (2 * D,), "Wi2": (D, 2 * D), "bi2": (D,),
    "gi": (D,), "bni": (D,),
    "Wc1": (D, NCI), "bc1": (D,), "Wc2": (D, D), "bc2": (D,),
    "gc": (D,), "bnc": (D,),
    "Wm": (D, 3 * D), "bm": (D,), "gm": (D,), "bnm": (D,),
    "Wk1": (D, D), "bk1": (D,), "Wk2": (NCLS, D), "bk2": (NCLS,),
}


def _ln(x, g, b, eps=1e-5):
    mu = x.mean(-1, keepdims=True)
    var = ((x - mu) ** 2).mean(-1, keepdims=True)
    return (x - mu) / jnp.sqrt(var + eps) * g + b


def _gelu(x):
    return jax.nn.gelu(x, approximate=False)


def _unpack_w(wflat):
    out = []
    off = 0
    for name in _WNAMES:
        shp = _WSHAPES[name]
        sz = int(np.prod(shp))
        out.append(wflat[off:off + sz].reshape(shp))
        off += sz
    return out


def _shard_fn(packed, wflat):
    (W_in, b_in, W_out, b_out, g_attn, b_attn,
     Wi1, bi1, Wi2, bi2, gi, bni,
     Wc1, bc1, Wc2, bc2, gc, bnc,
     Wm, bm, gm, bnm,
     Wk1, bk1, Wk2, bk2) = _unpack_w(wflat)

    n = packed.shape[0]
    hd = D // H
    e_vx = packed[:, 0 * D:1 * D]
    e_vy = packed[:, 1 * D:2 * D]
    e_xv = packed[:, 2 * D:3 * D]
    e_yv = packed[:, 3 * D:4 * D]
    ci_features = packed[:, 4 * D:4 * D + NCI]

    edges = jnp.stack([e_vx, e_vy, e_xv, e_yv], axis=1)        # (n,4,D)
    qkv = edges @ W_in.T + b_in                                # (n,4,3D)
    q, k, v = jnp.split(qkv, 3, axis=-1)
    sh = lambda t: t.reshape(n, 4, H, hd).transpose(0, 2, 1, 3)
    q, k, v = sh(q), sh(k), sh(v)
    scores = jnp.einsum("nhqe,nhke->nhqk", q, k) * (1.0 / hd ** 0.5)
    att = jax.nn.softmax(scores, axis=-1)
    ao = jnp.einsum("nhqk,nhke->nhqe", att, v).transpose(0, 2, 1, 3).reshape(n, 4, D)
    attended = ao @ W_out.T + b_out
    attended = _ln(edges + attended, g_attn, b_attn)
    pooled = attended.mean(axis=1)
    inter = jnp.concatenate(
        [e_vx * e_vy, e_vx * e_xv, e_vx * e_yv,
         e_vy * e_xv, e_vy * e_yv, e_xv * e_yv], axis=-1)
    interaction_emb = _ln(_gelu(inter @ Wi1.T + bi1) @ Wi2.T + bi2, gi, bni)
    ci_emb = _ln(_gelu(ci_features @ Wc1.T + bc1) @ Wc2.T + bc2, gc, bnc)
    merged = _gelu(_ln(
        jnp.concatenate([pooled, interaction_emb, ci_emb], axis=-1) @ Wm.T + bm,
        gm, bnm))
    return _gelu(merged @ Wk1.T + bk1) @ Wk2.T + bk2


_pmapped = None
_wcache = {}


def _get_pmapped():
    global _pmapped
    if _pmapped is None:
        _pmapped = jax.pmap(_shard_fn, devices=jax.devices()[:NDEV])
    return _pmapped


def kernel(**inputs):
    fn = _get_pmapped()

    # pack the five node tensors into one (NDEV, N/NDEV, 266) fp32 array
    packed = np.empty((N, PACKC), dtype=np.float32)
    packed[:, 0 * D:1 * D] = inputs["e_vx"]
    packed[:, 1 * D:2 * D] = inputs["e_vy"]
    packed[:, 2 * D:3 * D] = inputs["e_xv"]
    packed[:, 3 * D:4 * D] = inputs["e_yv"]
    packed[:, 4 * D:] = inputs["ci_features"]
    packed = packed.reshape(NDEV, N // NDEV, PACKC)

    # pack all weights into one flat replicated vector (cached upload)
    wkey = tuple(id(inputs[k]) for k in _WNAMES)
    wrep = _wcache.get(wkey)
    if wrep is None:
        wflat = np.concatenate(
            [np.asarray(inputs[k], dtype=np.float32).ravel() for k in _WNAMES])
        wrep = jax.device_put_replicated(wflat, jax.devices()[:NDEV])
        _wcache.clear()
        _wcache[wkey] = wrep

    out = fn(packed, wrep)
    return np.asarray(out).reshape(N, NCLS)
